# revision 1
# baseline (speedup 1.0000x reference)
"""Trainium2 Bass kernel for CohereAttention (QK-LayerNorm + interleaved RoPE +
GQA sliding-window attention), sharded over 8 NeuronCores.

Sharding: tensor-parallel over Q heads (4 per core); with H//KVH == 4 each core
owns exactly one KV head. Attention outputs are AllGathered (bf16) and o_proj is
column-parallel (512 output features per core), so no all-reduce is needed.

Device-side layouts are transposed ([feature, token]) so every matmul contracts
over the partition axis at full PE rate:
  - QK-LayerNorm mean subtraction is folded into the projection weights on the
    host (subtract per-head column mean), leaving an RMS-style normalization.
  - RoPE rotate-half is a partition pair-swap (DVE stream_shuffle) with the sign
    folded into the sin table on the host.
  - Scores are computed transposed (S^T[j, q]) so the PV matmul needs no
    transposes; softmax denominator comes from a ones-vector matmul and is
    applied at the attention-output drain.
  - Sliding-window/causal masks are applied with GPSIMD affine_select.
"""

import sys

sys.path.insert(0, "/opt/trn_rl_repo")

import numpy as np
import ml_dtypes

import concourse.bass as bass
import concourse.mybir as mybir
import concourse.tile as tile
from concourse import bacc
from concourse.bass import ts, ds
from concourse.bass_utils import run_bass_kernel_spmd

B, S, H, KVH, D, HID = 2, 2048, 32, 8, 128, 4096
WINDOW = 512
EPS = 1e-5
SCALE = float(D) ** -0.5
NC = 8
HPC = H // NC              # q heads per core (4)
QW = HPC * D               # q width per core (512)
OW = HID // NC             # o_proj output width per core (512)
FCH = HID // 128           # contraction chunks (32)
TT = 512                   # projection token tile
QT = 256                   # attention query tile
NKC = (WINDOW + QT) // 128  # key chunks per query tile window (6)

BF16 = mybir.dt.bfloat16
F32 = mybir.dt.float32
F32R = mybir.dt.float32r
npbf16 = ml_dtypes.bfloat16

SWAP32 = [i ^ 1 for i in range(32)]  # adjacent-pair partition swap

_CACHE = {}


def _build_module():
    nc = bacc.Bacc(
        "TRN2",
        target_bir_lowering=False,
        debug=False,
        enable_asserts=False,
        num_devices=NC,
    )

    hT = nc.dram_tensor("hT", [B, HID, S], BF16, kind="ExternalInput").ap()
    cosT = nc.dram_tensor("cosT", [B, D, S], BF16, kind="ExternalInput").ap()
    sinT = nc.dram_tensor("sinT", [B, D, S], BF16, kind="ExternalInput").ap()
    wq = nc.dram_tensor("wq", [HID, QW], BF16, kind="ExternalInput").ap()
    wk = nc.dram_tensor("wk", [HID, D], BF16, kind="ExternalInput").ap()
    wv = nc.dram_tensor("wv", [HID, D], BF16, kind="ExternalInput").ap()
    wo = nc.dram_tensor("wo", [HID, OW], BF16, kind="ExternalInput").ap()
    winvq = nc.dram_tensor("winvq", [D, 1], F32, kind="ExternalInput").ap()
    winvk = nc.dram_tensor("winvk", [D, 1], F32, kind="ExternalInput").ap()
    out = nc.dram_tensor("out", [B * S, OW], F32, kind="ExternalOutput").ap()

    agin = [
        nc.dram_tensor(f"agin{b}", [QW, S], BF16, kind="Internal").ap()
        for b in range(B)
    ]
    agout = [
        nc.dram_tensor(
            f"agout{b}", [HID, S], BF16, kind="Internal", addr_space="Shared"
        ).ap()
        for b in range(B)
    ]

    ident_d = nc.inline_tensor(np.eye(128, dtype=npbf16), name="ident").ap()
    ones_d = nc.inline_tensor(np.ones((128, 1), dtype=npbf16), name="onesv").ap()

    rg = [list(range(NC))]

    with tile.TileContext(nc) as tc, \
            tc.tile_pool(name="sb", bufs=1) as sb, \
            tc.tile_pool(name="ps", bufs=1, space="PSUM") as ps:

        # --- resident weights / constants ---
        wq_sb = sb.tile([128, FCH, QW], BF16, tag="wbig", bufs=1, name="wq_sb")
        nc.sync.dma_start(wq_sb[:], wq.rearrange("(c p) n -> p c n", p=128))
        wk_sb = sb.tile([128, FCH, D], BF16, tag="wk", bufs=1, name="wk_sb")
        nc.sync.dma_start(wk_sb[:], wk.rearrange("(c p) n -> p c n", p=128))
        wv_sb = sb.tile([128, FCH, D], BF16, tag="wv", bufs=1, name="wv_sb")
        nc.sync.dma_start(wv_sb[:], wv.rearrange("(c p) n -> p c n", p=128))
        ident_sb = sb.tile([128, 128], BF16, tag="ident", bufs=1, name="ident_sb")
        nc.sync.dma_start(ident_sb[:], ident_d)
        ones_sb = sb.tile([128, 1], BF16, tag="ones", bufs=1, name="ones_sb")
        nc.sync.dma_start(ones_sb[:], ones_d)
        winvq_sb = sb.tile([D, 1], F32, tag="winvq", bufs=1, name="winvq_sb")
        nc.sync.dma_start(winvq_sb[:], winvq)
        winvk_sb = sb.tile([D, 1], F32, tag="winvk", bufs=1, name="winvk_sb")
        nc.sync.dma_start(winvk_sb[:], winvk)
        eps_sb = sb.tile([1, 1], F32, tag="eps", bufs=1, name="eps_sb")
        nc.vector.memset(eps_sb[:], EPS)

        def ln_rope(qps, winv_sb, cos_sb, sin_sb, tt, dst):
            """LayerNorm (mean pre-folded) + interleaved RoPE on a transposed
            [d, TT] psum tile; writes bf16 into dst[:, tt*TT:...]."""
            sq = sb.tile([128, TT], F32, tag="sq", bufs=3, name="sq")
            nc.scalar.square(sq[:], qps[:])
            qsb = sb.tile([128, TT], F32, tag="qsb", bufs=3, name="qsb")
            nc.scalar.copy(qsb[:], qps[:])  # frees the psum bank early
            ssq = ps.tile([1, TT], F32, tag="misc", bufs=2, name="ssq")
            nc.tensor.matmul(ssq[:], winv_sb[:], sq[:], start=True, stop=True)
            std = sb.tile([1, TT], F32, tag="std", bufs=3, name="std")
            nc.scalar.activation(
                std[:], ssq[:], mybir.ActivationFunctionType.Sqrt,
                bias=eps_sb[:], scale=1.0 / D,
            )
            rstd = sb.tile([1, TT], F32, tag="rstd", bufs=3, name="rstd")
            nc.vector.reciprocal(rstd[:], std[:])
            rbc = sb.tile([128, TT], F32, tag="rbc", bufs=3, name="rbc")
            nc.gpsimd.partition_broadcast(rbc[:], rstd[:])
            qn = sb.tile([128, TT], BF16, tag="qn", bufs=3, name="qn")
            nc.vector.tensor_mul(qn[:], qsb[:], rbc[:])
            qs = sb.tile([128, TT], BF16, tag="qs", bufs=3, name="qs")
            nc.vector.stream_shuffle(qs[:], qn[:], SWAP32)
            t1 = sb.tile([128, TT], BF16, tag="t1", bufs=3, name="t1")
            nc.vector.tensor_mul(t1[:], qn[:], cos_sb[:, ts(tt, TT)])
            t2 = sb.tile([128, TT], BF16, tag="t2", bufs=3, name="t2")
            nc.vector.tensor_mul(t2[:], qs[:], sin_sb[:, ts(tt, TT)])
            nc.vector.tensor_add(dst[:, ts(tt, TT)], t1[:], t2[:])

        qT = {}   # (b, h) -> [128, S] bf16 rope'd normalized q, transposed
        kT = {}   # b -> [128, S]
        Vn = {}   # b -> [128, S] (natural [j, d] in 128-col chunks)
        vT = {}   # b -> [128, S] transposed v (pre PE-transpose)
        trig = {}  # b -> (cos_sb, sin_sb)

        def proj_setup(b):
            cos_sb = sb.tile([128, S], BF16, tag="cos", bufs=1, name="cos_sb")
            nc.sync.dma_start(cos_sb[:], cosT[b])
            sin_sb = sb.tile([128, S], BF16, tag="sin", bufs=1, name="sin_sb")
            nc.sync.dma_start(sin_sb[:], sinT[b])
            trig[b] = (cos_sb, sin_sb)
            for h in range(HPC):
                qT[(b, h)] = sb.tile([128, S], BF16, tag="qT", bufs=8,
                                     name=f"qT{b}{h}")
            kT[b] = sb.tile([128, S], BF16, tag="kT", bufs=2, name=f"kT{b}")
            vT[b] = sb.tile([128, S], BF16, tag="vT", bufs=1, name=f"vT{b}")
            Vn[b] = sb.tile([128, S], BF16, tag="Vn", bufs=2, name=f"Vn{b}")

        def proj_tt(b, tt):
            with nc.named_scope(f"proj_b{b}"):
                cos_sb, sin_sb = trig[b]
                qps = [
                    ps.tile([128, TT], F32, tag=f"acc{i}", bufs=1,
                            name=f"qps{i}")
                    for i in range(HPC)
                ]
                kps = ps.tile([128, TT], F32, tag="acck", bufs=1, name="kps")
                vps = ps.tile([128, TT], F32, tag="accv", bufs=1, name="vps")
                for f in range(FCH):
                    ht_t = sb.tile([128, TT], BF16, tag="ht", bufs=3,
                                   name="ht_t")
                    nc.sync.dma_start(
                        ht_t[:], hT[b, ds(f * 128, 128), ts(tt, TT)]
                    )
                    st = f == 0
                    sp = f == FCH - 1
                    for h in range(HPC):
                        nc.tensor.matmul(
                            qps[h][:], wq_sb[:, f, ts(h, D)], ht_t[:],
                            start=st, stop=sp,
                        )
                    nc.tensor.matmul(kps[:], wk_sb[:, f, :], ht_t[:],
                                     start=st, stop=sp)
                    nc.tensor.matmul(vps[:], wv_sb[:, f, :], ht_t[:],
                                     start=st, stop=sp)
                for h in range(HPC):
                    ln_rope(qps[h], winvq_sb, cos_sb, sin_sb, tt, qT[(b, h)])
                ln_rope(kps, winvk_sb, cos_sb, sin_sb, tt, kT[b])
                nc.scalar.copy(vT[b][:, ts(tt, TT)], vps[:])

        def proj_vtrans(b):
            # transpose v to natural [j, d] layout for the PV matmul
            with nc.named_scope(f"proj_b{b}"):
                for j in range(S // 128):
                    tp = ps.tile([128, 128], BF16, tag="misc", bufs=2, name="tp")
                    nc.tensor.transpose(tp[:], vT[b][:, ts(j, 128)], ident_sb[:])
                    nc.scalar.copy(Vn[b][:, ts(j, 128)], tp[:])

        def attn_head(b, h):
            with nc.named_scope(f"attn_b{b}"):
                if True:
                    attn_sb = sb.tile([128, S], BF16, tag="attn", bufs=2,
                                      name="attn_sb")
                    for qt in range(S // QT):
                        i0 = qt * QT
                        kstart = max(0, (WINDOW - i0) // 128)
                        ops = ps.tile([128, QT], F32, tag=f"acc{2 + qt % 2}",
                                      bufs=1, name="ops")
                        lps = ps.tile([1, QT], F32,
                                      tag="acck" if qt % 2 == 0 else "accv",
                                      bufs=1, name="lps")
                        for kk in range(kstart, NKC):
                            j0 = i0 - WINDOW + kk * 128
                            sps = ps.tile([128, QT], F32, tag=f"acc{kk % 2}",
                                          bufs=1, name="sps")
                            nc.tensor.matmul(
                                sps[:], kT[b][:, ds(j0, 128)],
                                qT[(b, h)][:, ds(i0, QT)],
                                start=True, stop=True,
                            )
                            pt = sb.tile([128, QT], BF16, tag="pt", bufs=4,
                                         name="pt")
                            nc.scalar.activation(
                                pt[:], sps[:], mybir.ActivationFunctionType.Exp,
                                scale=SCALE,
                            )
                            if kk * 128 < QT:  # window edge: keep iff u-1 >= 0
                                nc.gpsimd.affine_select(
                                    out=pt[:], in_=pt[:],
                                    compare_op=mybir.AluOpType.is_ge,
                                    fill=0.0, base=128 * kk - 1,
                                    channel_multiplier=1, pattern=[[-1, QT]],
                                )
                            elif kk * 128 > WINDOW - 128:  # causal edge: 512-u >= 0
                                nc.gpsimd.affine_select(
                                    out=pt[:], in_=pt[:],
                                    compare_op=mybir.AluOpType.is_ge,
                                    fill=0.0, base=WINDOW - 128 * kk,
                                    channel_multiplier=-1, pattern=[[1, QT]],
                                )
                            first = kk == kstart
                            last = kk == NKC - 1
                            nc.tensor.matmul(
                                ops[:], Vn[b][:, ds(j0, 128)], pt[:],
                                start=first, stop=last,
                            )
                            nc.tensor.matmul(
                                lps[:], ones_sb[:], pt[:],
                                start=first, stop=last,
                            )
                        linv = sb.tile([1, QT], F32, tag="linv", bufs=3,
                                       name="linv")
                        nc.vector.reciprocal(linv[:], lps[:])
                        lbc = sb.tile([128, QT], F32, tag="lbc", bufs=3,
                                      name="lbc")
                        nc.gpsimd.partition_broadcast(lbc[:], linv[:])
                        nc.vector.tensor_mul(attn_sb[:, ds(i0, QT)], ops[:],
                                             lbc[:])
                    nc.sync.dma_start(agin[b][ts(h, 128), :], attn_sb[:])

        def oproj_phase(b):
            with nc.named_scope(f"oproj_b{b}"):
                if b == 0:
                    wo_sb = _build_module.wo_sb = sb.tile(
                        [128, FCH, OW], BF16, tag="wbig", bufs=1, name="wo_sb"
                    )
                    nc.sync.dma_start(
                        wo_sb[:], wo.rearrange("(c p) n -> p c n", p=128)
                    )
                else:
                    wo_sb = _build_module.wo_sb
                agv = agout[b].rearrange("(c p) t -> p c t", p=128)
                for tq in range(S // 256):
                    og = sb.tile([128, FCH, 256], BF16, tag="og", bufs=2,
                                 name="og")
                    nc.sync.dma_start(og[:], agv[:, :, ts(tq, 256)])
                    for t2 in range(2):
                        po = ps.tile([128, OW], F32, tag="misc", bufs=2,
                                     name="po")
                        for c in range(FCH):
                            nc.tensor.matmul(
                                po[:], og[:, c, ts(t2, 128)], wo_sb[:, c, :],
                                start=(c == 0), stop=(c == FCH - 1),
                            )
                        ot = sb.tile([128, OW], F32, tag="ot", bufs=2, name="ot")
                        nc.scalar.copy(ot[:], po[:])
                        nc.sync.dma_start(
                            out[ds(b * S + tq * 256 + t2 * 128, 128), :], ot[:]
                        )

        def ag_phase(b):
            nc.gpsimd.collective_compute(
                "AllGather",
                mybir.AluOpType.bypass,
                replica_groups=rg,
                ins=[agin[b][:]],
                outs=[agout[b][:]],
            )

        proj_setup(0)
        for tt in range(S // TT):
            proj_tt(0, tt)
        proj_vtrans(0)
        proj_setup(1)
        # ladder: attention on b0 interleaved with projections for b1 so the
        # shared PSUM bank rings hand off per-rung instead of serializing
        for h in range(HPC):
            attn_head(0, h)
            proj_tt(1, h)
        ag_phase(0)
        proj_vtrans(1)
        for h in range(HPC):
            attn_head(1, h)
        oproj_phase(0)
        ag_phase(1)
        oproj_phase(1)

    nc.compile()
    return nc


def _prep_inputs(inputs):
    hidden = np.asarray(inputs["hidden_states"], np.float32)
    pos = np.asarray(inputs["position_ids"])
    cos = np.asarray(inputs["cos"], np.float32)
    sin = np.asarray(inputs["sin"], np.float32)
    wq = np.asarray(inputs["wq"], np.float32)
    wk = np.asarray(inputs["wk"], np.float32)
    wv = np.asarray(inputs["wv"], np.float32)
    wo = np.asarray(inputs["wo"], np.float32)
    qw = np.asarray(inputs["q_norm_w"], np.float32)
    kw = np.asarray(inputs["k_norm_w"], np.float32)

    hT = np.ascontiguousarray(hidden.transpose(0, 2, 1)).astype(npbf16)
    cosT = np.ascontiguousarray(cos[pos].transpose(0, 2, 1)).astype(npbf16)
    sinT_f = sin[pos].transpose(0, 2, 1).copy()
    sinT_f[:, 0::2, :] *= -1.0
    sinT = np.ascontiguousarray(sinT_f).astype(npbf16)

    winvq = (1.0 / np.where(qw == 0, 1, qw) ** 2).astype(np.float32).reshape(D, 1)
    winvk = (1.0 / np.where(kw == 0, 1, kw) ** 2).astype(np.float32).reshape(D, 1)

    in_maps = []
    for c in range(NC):
        wq_c = wq[:, c * QW:(c + 1) * QW].copy()
        for j in range(HPC):
            blk = wq_c[:, j * D:(j + 1) * D]
            blk -= blk.mean(axis=1, keepdims=True)
            blk *= qw[None, :]
        wk_c = wk[:, c * D:(c + 1) * D].copy()
        wk_c -= wk_c.mean(axis=1, keepdims=True)
        wk_c *= kw[None, :]
        in_maps.append({
            "hT": hT,
            "cosT": cosT,
            "sinT": sinT,
            "wq": np.ascontiguousarray(wq_c).astype(npbf16),
            "wk": np.ascontiguousarray(wk_c).astype(npbf16),
            "wv": np.ascontiguousarray(wv[:, c * D:(c + 1) * D]).astype(npbf16),
            "wo": np.ascontiguousarray(wo[:, c * OW:(c + 1) * OW]).astype(npbf16),
            "winvq": winvq,
            "winvk": winvk,
        })
    return in_maps


def _run(inputs, **kwargs):
    if "nc" not in _CACHE:
        _CACHE["nc"] = _build_module()
    nc = _CACHE["nc"]
    in_maps = _prep_inputs(inputs)
    res = run_bass_kernel_spmd(nc, in_maps, core_ids=list(range(NC)), **kwargs)
    shards = [res.results[c]["out"].reshape(B, S, OW) for c in range(NC)]
    return np.concatenate(shards, axis=-1).astype(np.float32), res


def kernel(**inputs) -> np.ndarray:
    out, _ = _run(inputs)
    return out


if __name__ == "__main__":
    import reference
    ins = {k: np.asarray(v) for k, v in reference.setup_inputs().items()}
    expected = np.asarray(reference.reference(**reference.setup_inputs()))
    actual = kernel(**ins)
    err = np.linalg.norm(actual - expected) / np.linalg.norm(expected)
    print("Relative error:", err)



# revision 8
# speedup vs baseline: 1.0293x; 1.0293x over previous
"""Trainium2 Bass kernel for CohereAttention (QK-LayerNorm + interleaved RoPE +
GQA sliding-window attention), sharded over 8 NeuronCores.

Sharding: tensor-parallel over Q heads (4 per core); with H//KVH == 4 each core
owns exactly one KV head. Attention outputs are AllGathered (bf16) and o_proj is
column-parallel (512 output features per core), so no all-reduce is needed.

Device-side layouts are transposed ([feature, token]) so every matmul contracts
over the partition axis at full PE rate:
  - QK-LayerNorm mean subtraction is folded into the projection weights on the
    host (subtract per-head column mean), leaving an RMS-style normalization.
  - RoPE rotate-half is a partition pair-swap (DVE stream_shuffle) with the sign
    folded into the sin table on the host.
  - Scores are computed transposed (S^T[j, q]) so the PV matmul needs no
    transposes; the softmax denominator comes from a ones-vector matmul.

Perf-critical design points (v2):
  - NO GpSimd work anywhere: the AllGather trigger instruction blocks the
    issuing engine queue until collective completion, so the collectives own
    the (otherwise empty) GpSimd queue and overlap freely with compute.
  - Sliding-window/causal masks are constant 0/1 tiles applied by DVE
    multiplies after the exp (replaces gpsimd affine_select).
  - Partition-broadcast of per-token scales is a K=1 PE matmul with a ones
    row (replaces gpsimd partition_broadcast).
  - 1/x via nc.vector.reciprocal_approx_fast (~5x faster than reciprocal).
  - Softmax epilogues are emitted one query-tile behind the score/PV matmuls
    so the PE queue never waits on the ACT/DVE chain.
  - Phase order overlaps AllGather(b0) with proj(b1) tt2/3 + attn(b1), and
    AllGather(b1) with o_proj(b0); weight/o_proj DMAs are split and
    prefetched just-in-time.
"""

import sys

sys.path.insert(0, "/opt/trn_rl_repo")

import numpy as np
import ml_dtypes

import concourse.bass as bass
import concourse.mybir as mybir
import concourse.tile as tile
from concourse import bacc
from concourse.bass import ts, ds
from concourse.bass_utils import run_bass_kernel_spmd

B, S, H, KVH, D, HID = 2, 2048, 32, 8, 128, 4096
WINDOW = 512
EPS = 1e-5
SCALE = float(D) ** -0.5
NC = 8
HPC = H // NC              # q heads per core (4)
QW = HPC * D               # q width per core (512)
OW = HID // NC             # o_proj output width per core (512)
FCH = HID // 128           # contraction chunks (32)
TT = 512                   # projection token tile
QT = 256                   # attention query tile
NKC = (WINDOW + QT) // 128  # key chunks per query tile window (6)

BF16 = mybir.dt.bfloat16
F32 = mybir.dt.float32
npbf16 = ml_dtypes.bfloat16
EXP = mybir.ActivationFunctionType.Exp
SQRT = mybir.ActivationFunctionType.Sqrt

SWAP32 = [i ^ 1 for i in range(32)]  # adjacent-pair partition swap

_CACHE = {}


def _mask_arrays():
    p = np.arange(128)[:, None]
    c128 = np.arange(128)[None, :]
    c256 = np.arange(QT)[None, :]
    m0 = (p >= c128 + 1).astype(npbf16)          # window edge, chunk kk=0
    m1 = (p >= c256 - 127).astype(npbf16)        # window edge, chunk kk=1
    m4 = (c256 >= p).astype(npbf16)              # causal edge, chunk kk=4
    return m0, m1, m4


def _build_module():
    nc = bacc.Bacc(
        "TRN2",
        target_bir_lowering=False,
        debug=False,
        enable_asserts=False,
        num_devices=NC,
    )

    hT = nc.dram_tensor("hT", [B, HID, S], BF16, kind="ExternalInput").ap()
    cosT = nc.dram_tensor("cosT", [B, D, S], BF16, kind="ExternalInput").ap()
    sinT = nc.dram_tensor("sinT", [B, D, S], BF16, kind="ExternalInput").ap()
    wq = nc.dram_tensor("wq", [HID, QW], BF16, kind="ExternalInput").ap()
    wk = nc.dram_tensor("wk", [HID, D], BF16, kind="ExternalInput").ap()
    wv = nc.dram_tensor("wv", [HID, D], BF16, kind="ExternalInput").ap()
    wo = nc.dram_tensor("wo", [HID, OW], BF16, kind="ExternalInput").ap()
    winvq = nc.dram_tensor("winvq", [D, 1], F32, kind="ExternalInput").ap()
    winvk = nc.dram_tensor("winvk", [D, 1], F32, kind="ExternalInput").ap()
    out = nc.dram_tensor("out", [B * S, OW], F32, kind="ExternalOutput").ap()

    agin = [
        nc.dram_tensor(f"agin{b}", [QW, S], BF16, kind="Internal").ap()
        for b in range(B)
    ]
    agout = [
        nc.dram_tensor(
            f"agout{b}", [HID, S], BF16, kind="Internal", addr_space="Shared"
        ).ap()
        for b in range(B)
    ]

    ident_d = nc.inline_tensor(np.eye(128, dtype=npbf16), name="ident").ap()
    onesc_d = nc.inline_tensor(np.ones((128, 1), dtype=npbf16), name="onesc").ap()
    onesr_d = nc.inline_tensor(np.ones((1, 128), dtype=npbf16), name="onesr").ap()
    m0_a, m1_a, m4_a = _mask_arrays()
    m0_d = nc.inline_tensor(m0_a, name="m0").ap()
    m1_d = nc.inline_tensor(m1_a, name="m1").ap()
    m4_d = nc.inline_tensor(m4_a, name="m4").ap()

    rg = [list(range(NC))]
    wq_r = wq.rearrange("(c p) n -> p c n", p=128)

    with tile.TileContext(nc) as tc, \
            tc.tile_pool(name="sb", bufs=1) as sb, \
            tc.tile_pool(name="ps", bufs=1, space="PSUM") as ps:

        # --- resident weights / constants ---
        # wq is DMA'd per-contraction-chunk (just-in-time, inside the first
        # proj token-tile loop) so the first matmuls start early.
        wq_sb = sb.tile([128, FCH, QW], BF16, tag="wbig", bufs=1, name="wq_sb")
        wk_sb = sb.tile([128, FCH, D], BF16, tag="wk", bufs=1, name="wk_sb")
        nc.sync.dma_start(wk_sb[:], wk.rearrange("(c p) n -> p c n", p=128))
        wv_sb = sb.tile([128, FCH, D], BF16, tag="wv", bufs=1, name="wv_sb")
        nc.sync.dma_start(wv_sb[:], wv.rearrange("(c p) n -> p c n", p=128))
        ident_sb = sb.tile([128, 128], BF16, tag="ident", bufs=1, name="ident_sb")
        nc.sync.dma_start(ident_sb[:], ident_d)
        onesc_sb = sb.tile([128, 1], BF16, tag="onesc", bufs=1, name="onesc_sb")
        nc.sync.dma_start(onesc_sb[:], onesc_d)
        onesr_sb = sb.tile([1, 128], BF16, tag="onesr", bufs=1, name="onesr_sb")
        nc.sync.dma_start(onesr_sb[:], onesr_d)
        m0_sb = sb.tile([128, 128], BF16, tag="m0", bufs=1, name="m0_sb")
        nc.sync.dma_start(m0_sb[:], m0_d)
        m1_sb = sb.tile([128, QT], BF16, tag="m1", bufs=1, name="m1_sb")
        nc.sync.dma_start(m1_sb[:], m1_d)
        m4_sb = sb.tile([128, QT], BF16, tag="m4", bufs=1, name="m4_sb")
        nc.sync.dma_start(m4_sb[:], m4_d)
        winvq_sb = sb.tile([D, 1], F32, tag="winvq", bufs=1, name="winvq_sb")
        nc.sync.dma_start(winvq_sb[:], winvq)
        winvk_sb = sb.tile([D, 1], F32, tag="winvk", bufs=1, name="winvk_sb")
        nc.sync.dma_start(winvk_sb[:], winvk)
        eps_sb = sb.tile([1, 1], F32, tag="eps", bufs=1, name="eps_sb")
        nc.vector.memset(eps_sb[:], EPS)

        qT = {}   # (b, h) -> [128, S] bf16 rope'd normalized q, transposed
        kT = {}   # b -> [128, S]
        Vn = {}   # b -> [128, S] (natural [j, d] in 128-col chunks)
        vT = {}   # b -> [128, S] transposed v (pre PE-transpose)
        trig = {}  # b -> (cos_sb, sin_sb)

        def proj_setup(b):
            cos_sb = sb.tile([128, S], BF16, tag="cos", bufs=1, name="cos_sb")
            nc.sync.dma_start(cos_sb[:], cosT[b])
            sin_sb = sb.tile([128, S], BF16, tag="sin", bufs=1, name="sin_sb")
            nc.sync.dma_start(sin_sb[:], sinT[b])
            trig[b] = (cos_sb, sin_sb)
            for h in range(HPC):
                qT[(b, h)] = sb.tile([128, S], BF16, tag="qT", bufs=8,
                                     name=f"qT{b}{h}")
            kT[b] = sb.tile([128, S], BF16, tag="kT", bufs=2, name=f"kT{b}")
            vT[b] = sb.tile([128, S], BF16, tag="vT", bufs=1, name=f"vT{b}")
            Vn[b] = sb.tile([128, S], BF16, tag="Vn", bufs=2, name=f"Vn{b}")

        def bcast_cols(src_sb, width):
            """Broadcast a [1, width] bf16 row across 128 partitions via a
            K=1 PE matmul; returns an f32 SBUF tile [128, width]."""
            bc_ps = ps.tile([128, width], F32, tag="misc", bufs=2, name="bc_ps")
            nc.tensor.matmul(bc_ps[:], onesr_sb[:], src_sb[:],
                             start=True, stop=True)
            bc = sb.tile([128, width], F32, tag=f"bc{width}", bufs=2, name="bc")
            nc.vector.tensor_scalar_mul(bc[:], bc_ps[:], 1.0)
            return bc

        def proj_tt(b, tt):
            with nc.named_scope(f"proj_b{b}"):
                cos_sb, sin_sb = trig[b]
                qps = [
                    ps.tile([128, TT], F32, tag=f"acc{i}", bufs=1,
                            name=f"qps{i}")
                    for i in range(HPC)
                ]
                kps = ps.tile([128, TT], F32, tag="acck", bufs=1, name="kps")
                vps = ps.tile([128, TT], F32, tag="accv", bufs=1, name="vps")
                for f in range(FCH):
                    if b == 0 and tt == 0:
                        nc.sync.dma_start(wq_sb[:, f, :], wq_r[:, f, :])
                    ht_t = sb.tile([128, TT], BF16, tag="ht", bufs=3,
                                   name="ht_t")
                    nc.sync.dma_start(
                        ht_t[:], hT[b, ds(f * 128, 128), ts(tt, TT)]
                    )
                    st = f == 0
                    sp = f == FCH - 1
                    for h in range(HPC):
                        nc.tensor.matmul(
                            qps[h][:], wq_sb[:, f, ts(h, D)], ht_t[:],
                            start=st, stop=sp,
                        )
                    nc.tensor.matmul(kps[:], wk_sb[:, f, :], ht_t[:],
                                     start=st, stop=sp)
                    nc.tensor.matmul(vps[:], wv_sb[:, f, :], ht_t[:],
                                     start=st, stop=sp)

                # phase A: drain all PSUM accumulators quickly (ACT) so the
                # next token-tile's matmuls can reuse the banks immediately.
                units = []  # (sq, qsrc, winv, dst) in drain order

                def phase_a(ps_tile, winv_sb, dst):
                    sq = sb.tile([128, TT], F32, tag="sq", bufs=2, name="sq")
                    nc.scalar.square(sq[:], ps_tile[:])
                    qsb = sb.tile([128, TT], BF16, tag="qsb", bufs=5,
                                  name="qsb")
                    nc.scalar.copy(qsb[:], ps_tile[:])
                    units.append((sq, qsb, winv_sb, dst))

                phase_a(qps[0], winvq_sb, qT[(b, 0)])
                nc.scalar.copy(vT[b][:, ts(tt, TT)], vps[:])
                phase_a(kps, winvk_sb, kT[b])
                for h in range(1, HPC):
                    phase_a(qps[h], winvq_sb, qT[(b, h)])

                # sum-of-squares matmuls (PE), then rstd chains (ACT/DVE)
                ssqs = []
                for sq, _, winv_sb, _ in units:
                    ssq = ps.tile([1, TT], F32, tag="misc", bufs=2, name="ssq")
                    nc.tensor.matmul(ssq[:], winv_sb[:], sq[:],
                                     start=True, stop=True)
                    ssqs.append(ssq)
                rstdbs = []
                for ssq in ssqs:
                    std = sb.tile([1, TT], F32, tag="std", bufs=2, name="std")
                    nc.scalar.activation(std[:], ssq[:], SQRT,
                                         bias=eps_sb[:], scale=1.0 / D)
                    rstd = sb.tile([1, TT], F32, tag="rstd", bufs=2,
                                   name="rstd")
                    nc.vector.reciprocal_approx_fast(rstd[:], std[:])
                    rstdb = sb.tile([1, TT], BF16, tag="rstdb", bufs=5,
                                    name="rstdb")
                    nc.scalar.copy(rstdb[:], rstd[:])
                    rstdbs.append(rstdb)

                # normalization + interleaved RoPE per unit
                for (_, qsb, _, dst), rstdb in zip(units, rstdbs):
                    rbc = bcast_cols(rstdb, TT)
                    qn = sb.tile([128, TT], BF16, tag="qn", bufs=2, name="qn")
                    nc.vector.tensor_mul(qn[:], qsb[:], rbc[:])
                    qs = sb.tile([128, TT], BF16, tag="qs", bufs=2, name="qs")
                    nc.vector.stream_shuffle(qs[:], qn[:], SWAP32)
                    t1 = sb.tile([128, TT], BF16, tag="t1", bufs=2, name="t1")
                    nc.vector.tensor_mul(t1[:], qn[:], cos_sb[:, ts(tt, TT)])
                    t2 = sb.tile([128, TT], BF16, tag="t2", bufs=2, name="t2")
                    nc.vector.tensor_mul(t2[:], qs[:], sin_sb[:, ts(tt, TT)])
                    nc.vector.tensor_add(dst[:, ts(tt, TT)], t1[:], t2[:])

        def vtrans_tt(b, tt):
            # transpose this token-tile of v to natural [j, d] layout
            with nc.named_scope(f"proj_b{b}"):
                for j in range(tt * 4, tt * 4 + 4):
                    tp = ps.tile([128, 128], BF16, tag="misc", bufs=2,
                                 name="tp")
                    nc.tensor.transpose(tp[:], vT[b][:, ts(j, 128)],
                                        ident_sb[:])
                    nc.scalar.copy(Vn[b][:, ts(j, 128)], tp[:])

        def attn_epilogue(b, h, qt, ops, lps, attn_sb):
            i0 = qt * QT
            linv = sb.tile([1, QT], F32, tag="linv", bufs=3, name="linv")
            nc.vector.reciprocal_approx_fast(linv[:], lps[:])
            linvb = sb.tile([1, QT], BF16, tag="linvb", bufs=3, name="linvb")
            nc.scalar.copy(linvb[:], linv[:])
            rbc = bcast_cols(linvb, QT)
            nc.vector.tensor_mul(attn_sb[:, ds(i0, QT)], ops[:], rbc[:])

        def attn_head(b, h):
            with nc.named_scope(f"attn_b{b}"):
                attn_sb = sb.tile([128, S], BF16, tag="attn", bufs=2,
                                  name="attn_sb")
                pend = None  # deferred epilogue: (qt, ops, lps)
                for qt in range(S // QT):
                    i0 = qt * QT
                    kstart = max(0, (WINDOW - i0) // 128)
                    ops = ps.tile([128, QT], F32, tag=f"acc{2 + qt % 2}",
                                  bufs=1, name="ops")
                    lps = ps.tile([1, QT], F32,
                                  tag="acck" if qt % 2 == 0 else "accv",
                                  bufs=1, name="lps")
                    ls = sb.tile([128, QT], BF16, tag="ls", bufs=3, name="ls")
                    ls_init = [False, False]  # [0:128], [128:256] halves
                    # The PV accumulation group must start with a full-width
                    # matmul (the simulator's PSUM model rejects mixed
                    # written/fresh regions), so chunk 0's half-width PV is
                    # deferred until right after the first full chunk's PV.
                    pv_start_kk = 1 if kstart == 0 else kstart
                    pend_pv = None
                    for kk in range(kstart, NKC):
                        j0 = i0 - WINDOW + kk * 128
                        if kk == 0:
                            lo, hi = 0, 128
                        elif kk == NKC - 1:
                            lo, hi = 128, QT
                        else:
                            lo, hi = 0, QT
                        w = hi - lo
                        sps = ps.tile([128, QT], F32, tag=f"acc{kk % 2}",
                                      bufs=1, name="sps")
                        nc.tensor.matmul(
                            sps[:, ds(lo, w)], kT[b][:, ds(j0, 128)],
                            qT[(b, h)][:, ds(i0 + lo, w)],
                            start=True, stop=True,
                        )
                        pt = sb.tile([128, QT], BF16, tag="pt", bufs=4,
                                     name="pt")
                        nc.scalar.activation(pt[:, ds(lo, w)],
                                             sps[:, ds(lo, w)], EXP,
                                             scale=SCALE)
                        if kk == 0:
                            nc.vector.tensor_mul(pt[:, ds(0, 128)],
                                                 pt[:, ds(0, 128)], m0_sb[:])
                        elif kk == 1:
                            nc.vector.tensor_mul(pt[:], pt[:], m1_sb[:])
                        elif kk == 4:
                            nc.vector.tensor_mul(pt[:], pt[:], m4_sb[:])
                        elif kk == 5:
                            nc.vector.tensor_mul(pt[:, ds(128, 128)],
                                                 pt[:, ds(128, 128)],
                                                 m4_sb[:, ds(0, 128)])
                        if kk == 0:
                            pend_pv = (j0, pt)
                        else:
                            nc.tensor.matmul(
                                ops[:, ds(lo, w)], Vn[b][:, ds(j0, 128)],
                                pt[:, ds(lo, w)],
                                start=(kk == pv_start_kk),
                                stop=(kk == NKC - 1),
                                skip_group_check=True,
                            )
                            if pend_pv is not None:
                                pj0, ppt = pend_pv
                                pend_pv = None
                                nc.tensor.matmul(
                                    ops[:, ds(0, 128)],
                                    Vn[b][:, ds(pj0, 128)], ppt[:, ds(0, 128)],
                                    start=False, stop=False,
                                    skip_group_check=True,
                                )
                        # accumulate exp sums on DVE (frees PE of ones-MMs)
                        if lo == 0 and hi == QT and all(ls_init):
                            nc.vector.tensor_add(ls[:], ls[:], pt[:])
                        else:
                            for half, (a0, a1) in enumerate(((0, 128),
                                                            (128, QT))):
                                s0, s1 = max(a0, lo), min(a1, hi)
                                if s0 >= s1:
                                    continue
                                sw = s1 - s0
                                if ls_init[half]:
                                    nc.vector.tensor_add(
                                        ls[:, ds(s0, sw)], ls[:, ds(s0, sw)],
                                        pt[:, ds(s0, sw)])
                                else:
                                    nc.vector.tensor_scalar_mul(
                                        ls[:, ds(s0, sw)], pt[:, ds(s0, sw)],
                                        1.0)
                                    ls_init[half] = True
                    nc.tensor.matmul(lps[:], onesc_sb[:], ls[:],
                                     start=True, stop=True)
                    # emit the previous tile's epilogue now: its inputs are
                    # long ready, so the PE queue never waits on ACT/DVE.
                    if pend is not None:
                        attn_epilogue(b, h, pend[0], pend[1], pend[2],
                                      attn_sb)
                    pend = (qt, ops, lps)
                attn_epilogue(b, h, pend[0], pend[1], pend[2], attn_sb)
                nc.sync.dma_start(agin[b][ts(h, 128), :], attn_sb[:])

        og_pend = {}

        def oproj_load(b, tq):
            agv = agout[b].rearrange("(c p) t -> p c t", p=128)
            og = sb.tile([128, FCH, 256], BF16, tag="og", bufs=2, name="og")
            nc.sync.dma_start(og[:], agv[:, :, ts(tq, 256)])
            og_pend[(b, tq)] = og

        def oproj_compute(b, tq):
            with nc.named_scope(f"oproj_b{b}"):
                og = og_pend.pop((b, tq))
                for t2 in range(2):
                    po = ps.tile([128, OW], F32, tag="misc", bufs=2,
                                 name="po")
                    for c in range(FCH):
                        nc.tensor.matmul(
                            po[:], og[:, c, ts(t2, 128)], wo_sb[:, c, :],
                            start=(c == 0), stop=(c == FCH - 1),
                        )
                    ot = sb.tile([128, OW], F32, tag="ot", bufs=2, name="ot")
                    nc.scalar.copy(ot[:], po[:])
                    nc.sync.dma_start(
                        out[ds(b * S + tq * 256 + t2 * 128, 128), :], ot[:]
                    )

        def ag_phase(b):
            nc.gpsimd.collective_compute(
                "AllGather",
                mybir.AluOpType.bypass,
                replica_groups=rg,
                ins=[agin[b][:]],
                outs=[agout[b][:]],
            )

        # ---- phase schedule ----
        proj_setup(0)
        for tt in range(S // TT):
            proj_tt(0, tt)
            vtrans_tt(0, tt)
        proj_setup(1)
        # ladder: attention b0 interleaved with the first half of proj b1
        attn_head(0, 0)
        proj_tt(1, 0)
        vtrans_tt(1, 0)
        attn_head(0, 1)
        proj_tt(1, 1)
        vtrans_tt(1, 1)
        attn_head(0, 2)
        attn_head(0, 3)
        ag_phase(0)
        # overlapped with AllGather(b0): rest of proj b1 + attention b1
        proj_tt(1, 2)
        vtrans_tt(1, 2)
        proj_tt(1, 3)
        vtrans_tt(1, 3)
        attn_head(1, 0)
        # prefetch o_proj weights + first activation tiles during attn b1
        wo_sb = sb.tile([128, FCH, OW], BF16, tag="wbig", bufs=1, name="wo_sb")
        wo_r = wo.rearrange("(c p) n -> p c n", p=128)
        for f8 in range(4):
            nc.sync.dma_start(wo_sb[:, ds(f8 * 8, 8), :],
                              wo_r[:, ds(f8 * 8, 8), :])
        oproj_load(0, 0)
        oproj_load(0, 1)
        attn_head(1, 1)
        attn_head(1, 2)
        attn_head(1, 3)
        ag_phase(1)
        # o_proj for both batches with lookahead-2 activation prefetch
        seq = [(0, tq) for tq in range(8)] + [(1, tq) for tq in range(8)]
        for i, (b, tq) in enumerate(seq):
            if i + 2 < len(seq):
                oproj_load(*seq[i + 2])
            oproj_compute(b, tq)

    nc.compile()
    return nc


def _prep_inputs(inputs):
    hidden = np.asarray(inputs["hidden_states"], np.float32)
    pos = np.asarray(inputs["position_ids"])
    cos = np.asarray(inputs["cos"], np.float32)
    sin = np.asarray(inputs["sin"], np.float32)
    wq = np.asarray(inputs["wq"], np.float32)
    wk = np.asarray(inputs["wk"], np.float32)
    wv = np.asarray(inputs["wv"], np.float32)
    wo = np.asarray(inputs["wo"], np.float32)
    qw = np.asarray(inputs["q_norm_w"], np.float32)
    kw = np.asarray(inputs["k_norm_w"], np.float32)

    hT = np.ascontiguousarray(hidden.transpose(0, 2, 1)).astype(npbf16)
    cosT = np.ascontiguousarray(cos[pos].transpose(0, 2, 1)).astype(npbf16)
    sinT_f = sin[pos].transpose(0, 2, 1).copy()
    sinT_f[:, 0::2, :] *= -1.0
    sinT = np.ascontiguousarray(sinT_f).astype(npbf16)

    winvq = (1.0 / np.where(qw == 0, 1, qw) ** 2).astype(np.float32).reshape(D, 1)
    winvk = (1.0 / np.where(kw == 0, 1, kw) ** 2).astype(np.float32).reshape(D, 1)

    in_maps = []
    for c in range(NC):
        wq_c = wq[:, c * QW:(c + 1) * QW].copy()
        for j in range(HPC):
            blk = wq_c[:, j * D:(j + 1) * D]
            blk -= blk.mean(axis=1, keepdims=True)
            blk *= qw[None, :]
        wk_c = wk[:, c * D:(c + 1) * D].copy()
        wk_c -= wk_c.mean(axis=1, keepdims=True)
        wk_c *= kw[None, :]
        in_maps.append({
            "hT": hT,
            "cosT": cosT,
            "sinT": sinT,
            "wq": np.ascontiguousarray(wq_c).astype(npbf16),
            "wk": np.ascontiguousarray(wk_c).astype(npbf16),
            "wv": np.ascontiguousarray(wv[:, c * D:(c + 1) * D]).astype(npbf16),
            "wo": np.ascontiguousarray(wo[:, c * OW:(c + 1) * OW]).astype(npbf16),
            "winvq": winvq,
            "winvk": winvk,
        })
    return in_maps


def _run(inputs, **kwargs):
    if "nc" not in _CACHE:
        _CACHE["nc"] = _build_module()
    nc = _CACHE["nc"]
    in_maps = _prep_inputs(inputs)
    res = run_bass_kernel_spmd(nc, in_maps, core_ids=list(range(NC)), **kwargs)
    shards = [res.results[c]["out"].reshape(B, S, OW) for c in range(NC)]
    return np.concatenate(shards, axis=-1).astype(np.float32), res


def kernel(**inputs) -> np.ndarray:
    out, _ = _run(inputs)
    return out


if __name__ == "__main__":
    import reference
    ins = {k: np.asarray(v) for k, v in reference.setup_inputs().items()}
    expected = np.asarray(reference.reference(**reference.setup_inputs()))
    actual = kernel(**ins)
    err = np.linalg.norm(actual - expected) / np.linalg.norm(expected)
    print("Relative error:", err)
